# revision 1
# baseline (speedup 1.0000x reference)
"""Multi-head attention (RoPE + mask + softmax) Trainium2 Bass kernel.

Sharding: 8 cores = 2 batches x 4 head-groups. Core c handles batch c//4,
local heads 4*(c%4) .. +4 (tensor-parallel on heads; Wq/Wk/Wv column-sharded,
Wo row-sharded; per-core partial outputs summed on host).

All DRAM inputs are host-pre-tiled so every DMA is partition-contiguous
(~128 descriptors). Per-core pipeline (S=2048, 4 heads of dim 64):
  qhT/khT = (Wq_perm)^T @ q^T   [2x128, 2048] f32r   (PE, K=1024 accum)
  RoPE fused into psum eviction: t = psum*cos, u = psum*sin_signed (DVE),
    swap 32-row blocks of u via SBUF->SBUF DMA (gpsimd queue), add (DVE)
  vh = v @ Wv  [2048, 4*65] bf16 with ones column per head (PE + strided ACT evict)
  per (q-block 1024, head-pair, k-chunk, head): scoresT[k,q] (PE, K=64),
    exp(x/8) (ACT psum->bf16), mask-mul (DVE bf16),
    attn@V accumulate [65, 1024] (PE bf16; row 64 = softmax denominator)
  denominators per (qb, head-pair): reciprocal_approx_accurate on a [128, 16]
    reshape (DRAM bounce), PE K=1 ones-broadcast, DVE normalize -> outT f32r
  out_part = outT^T @ Wo  (PE, 4x K=64 accum) -> [2048, 1024] f32
"""
import sys
sys.path.insert(0, '/opt/trn_rl_repo')
import math
import numpy as np
import ml_dtypes

import concourse.bass as bass
import concourse.mybir as mybir
import concourse.tile as tile
from concourse import bacc
from concourse.bass_utils import run_bass_kernel_spmd

F32 = mybir.dt.float32
F32R = mybir.dt.float32r
BF16 = mybir.dt.bfloat16

S = 2048
DIM = 1024
HEAD_DIM = 64
N_CORES = 8
KC = DIM // 128          # 8 contraction chunks for projections
MT = S // 128            # 16 k-chunks in attention
QB = 1024                # q-block width
NQB = S // QB            # 2
ROPE_THETA = 10000.0

_BUILT = None


def build_bass():
    nc = bacc.Bacc("TRN2", target_bir_lowering=False, debug=False)

    qT = nc.dram_tensor("qT", [4, 128, KC, 512], F32R, kind="ExternalInput").ap()
    kT = nc.dram_tensor("kT", [4, 128, KC, 512], F32R, kind="ExternalInput").ap()
    vT = nc.dram_tensor("vT", [MT, 128, KC, 128], F32R, kind="ExternalInput").ap()
    wq = nc.dram_tensor("wq", [128, KC, 256], F32R, kind="ExternalInput").ap()
    wk = nc.dram_tensor("wk", [128, KC, 256], F32R, kind="ExternalInput").ap()
    wv = nc.dram_tensor("wv", [128, KC, 256], F32R, kind="ExternalInput").ap()
    wo = nc.dram_tensor("wo", [64, 4, DIM], F32R, kind="ExternalInput").ap()
    cosT = nc.dram_tensor("cosT", [128, S], F32, kind="ExternalInput").ap()
    sinT = nc.dram_tensor("sinT", [128, S], F32, kind="ExternalInput").ap()
    maskT = nc.dram_tensor("maskT", [128, MT, S], BF16, kind="ExternalInput").ap()
    ones64 = nc.dram_tensor("ones64", [1, 64], F32R, kind="ExternalInput").ap()
    out_part = nc.dram_tensor("out_part", [S, DIM], F32, kind="ExternalOutput").ap()

    with tile.TileContext(nc) as tc:
        with tc.tile_pool(name="persist", bufs=1) as persist, \
             tc.tile_pool(name="dram", bufs=1, space="DRAM") as dram, \
             tc.tile_pool(name="ps", bufs=4, space="PSUM") as ps:

            qhT = persist.tile([128, 2, S], F32R)     # [chunk-part, chunk, s]
            khT = persist.tile([128, 2, S], F32R)
            vh = persist.tile([128, MT, 4 * 65], BF16)
            outT = persist.tile([64, 4, S], F32R)
            wo_sb = persist.tile([64, 4, DIM], F32R)
            ones_sb = persist.tile([1, 64], F32R)
            dscr = dram.tile([8, QB], F32)
            dscr2 = dram.tile([8, QB], F32R)

            nc.sync.dma_start(out=wo_sb, in_=wo)
            nc.sync.dma_start(out=ones_sb, in_=ones64)
            # ones column for the denominator rows of vh
            nc.vector.memset(
                vh.rearrange("p m (h x) -> p m h x", x=65)[:, :, :, 64:65], 1.0)

            # ---------------- Phase 1+2: projections + RoPE ----------------
            with tc.tile_pool(name="proj", bufs=1) as projp, \
                 tc.tile_pool(name="xts", bufs=2) as xts, \
                 tc.tile_pool(name="rope", bufs=2) as rope:
                wq_sb = projp.tile([128, KC, 256], F32R)
                wk_sb = projp.tile([128, KC, 256], F32R)
                wv_sb = projp.tile([128, KC, 256], F32R)
                cos_sb = projp.tile([128, S], F32)
                sin_sb = projp.tile([128, S], F32)
                nc.sync.dma_start(out=wq_sb, in_=wq)
                nc.sync.dma_start(out=wk_sb, in_=wk)
                nc.sync.dma_start(out=wv_sb, in_=wv)
                nc.sync.dma_start(out=cos_sb, in_=cosT)
                nc.sync.dma_start(out=sin_sb, in_=sinT)

                # q/k projections with fused RoPE eviction
                for xdram, w_sb, dstT in ((qT, wq_sb, qhT), (kT, wk_sb, khT)):
                    for sblk in range(4):
                        x_sb = xts.tile([128, KC, 512], F32R, tag="xts")
                        nc.sync.dma_start(out=x_sb, in_=xdram[sblk])
                        ss = slice(sblk * 512, (sblk + 1) * 512)
                        for m in range(2):
                            psum = ps.tile([128, QB], F32, tag="ps")
                            for kc in range(KC):
                                nc.tensor.matmul(
                                    psum[:, 0:512],
                                    lhsT=w_sb[:, kc, m * 128:(m + 1) * 128],
                                    rhs=x_sb[:, kc, :],
                                    start=(kc == 0), stop=(kc == KC - 1))
                            t = rope.tile([128, 512], F32, tag="t")
                            u = rope.tile([128, 512], F32, tag="u")
                            nc.vector.tensor_mul(t, psum[:, 0:512], cos_sb[:, ss])
                            nc.vector.tensor_mul(u, psum[:, 0:512], sin_sb[:, ss])
                            us = rope.tile([128, 512], F32, tag="us")
                            for blk in range(4):
                                a, b2 = blk * 32, (blk ^ 1) * 32
                                nc.gpsimd.dma_start(out=us[a:a + 32, :],
                                                    in_=u[b2:b2 + 32, :])
                            nc.vector.tensor_add(dstT[:, m, ss], t, us)

                # v projection with strided bf16 eviction (+ ones cols preset)
                for sc in range(MT):
                    v_sb = xts.tile([128, KC, 128], F32R, tag="xts")
                    nc.sync.dma_start(out=v_sb, in_=vT[sc])
                    psum = ps.tile([128, QB], F32, tag="ps")
                    for kc in range(KC):
                        nc.tensor.matmul(
                            psum[:, 0:256], lhsT=v_sb[:, kc, :], rhs=wv_sb[:, kc, :],
                            start=(kc == 0), stop=(kc == KC - 1))
                    nc.scalar.copy(
                        vh[:, sc, :].rearrange("p (h x) -> p h x", x=65)[:, :, 0:64],
                        psum[:, 0:256].rearrange("p (h x) -> p h x", x=64))

            # ---------------- Phase 3: attention ----------------
            with tc.tile_pool(name="mask", bufs=1) as maskp, \
                 tc.tile_pool(name="attn", bufs=3) as attnp, \
                 tc.tile_pool(name="dn", bufs=2) as dnp:
                mk = maskp.tile([128, MT, S], BF16, tag="mask")
                for mq in range(4):
                    nc.sync.dma_start(out=mk[:, mq * 4:(mq + 1) * 4, :],
                                      in_=maskT[:, mq * 4:(mq + 1) * 4, :])
                stg = dnp.tile([128, QB], F32, tag="stg")
                for qb in range(NQB):
                    qs = slice(qb * QB, (qb + 1) * QB)
                    for hp in range(2):
                        avp = [ps.tile([128, QB], F32, tag="ps", name=f"avp{_i}")
                               for _i in range(2)]
                        for m in range(MT):
                            for h2 in range(2):
                                hb = slice(h2 * 64, (h2 + 1) * 64)
                                sps = ps.tile([128, QB], F32, tag="ps")
                                for q2 in range(2):
                                    q5 = slice(q2 * 512, (q2 + 1) * 512)
                                    nc.tensor.matmul(
                                        sps[:, q5],
                                        lhsT=khT[hb, hp, m * 128:(m + 1) * 128],
                                        rhs=qhT[hb, hp, qb * QB + q2 * 512:
                                                qb * QB + (q2 + 1) * 512],
                                        start=True, stop=True)
                                at = attnp.tile([128, QB], BF16, tag="at")
                                nc.scalar.activation(
                                    at, sps, mybir.ActivationFunctionType.Exp,
                                    scale=1.0 / math.sqrt(HEAD_DIM))
                                atm = attnp.tile([128, QB], BF16, tag="atm")
                                nc.vector.tensor_mul(atm, at, mk[:, m, qs])
                                h = 2 * hp + h2
                                for q2 in range(2):
                                    q5 = slice(q2 * 512, (q2 + 1) * 512)
                                    nc.tensor.matmul(
                                        avp[h2][0:65, q5],
                                        lhsT=vh[:, m, h * 65:(h + 1) * 65],
                                        rhs=atm[:, q5],
                                        start=(m == 0), stop=(m == MT - 1))
                        # evict + normalize this (qb, head-pair) right away
                        for h2 in range(2):
                            h = 2 * hp + h2
                            unit = qb * 4 + hp * 2 + h2
                            nc.vector.tensor_copy(outT[0:64, h, qs], avp[h2][0:64, :])
                            nc.scalar.copy(stg[64:65, :], avp[h2][64:65, :])
                            nc.sync.dma_start(out=dscr[unit, :], in_=stg[64:65, :])
                        u0 = qb * 4 + hp * 2
                        rin = dnp.tile([128, 2, 8], F32, tag="rin")
                        nc.sync.dma_start(
                            out=rin,
                            in_=dscr[u0:u0 + 2].rearrange("u (p f) -> p u f", p=128))
                        r32 = dnp.tile([128, 2, 8], F32, tag="r32")
                        scr = dnp.tile([128, 2, 8], F32, tag="scr")
                        nc.vector.reciprocal_approx_accurate(r32, rin, scr)
                        rr = dnp.tile([128, 2, 8], F32R, tag="rr")
                        nc.vector.tensor_copy(rr, r32)
                        nc.sync.dma_start(
                            out=dscr2[u0:u0 + 2].rearrange("u (p f) -> p u f", p=128),
                            in_=rr)
                        for h2 in range(2):
                            h = 2 * hp + h2
                            unit = u0 + h2
                            rdn = dnp.tile([1, QB], F32R, tag="rdn")
                            nc.sync.dma_start(out=rdn, in_=dscr2[unit:unit + 1, :])
                            pbc = ps.tile([128, QB], F32, tag="ps")
                            for q2 in range(2):
                                q5 = slice(q2 * 512, (q2 + 1) * 512)
                                nc.tensor.matmul(pbc[0:64, q5], lhsT=ones_sb,
                                                 rhs=rdn[:, q5], start=True, stop=True)
                            nc.vector.tensor_mul(outT[0:64, h, qs],
                                                 outT[0:64, h, qs], pbc[0:64, :])

            # ---------------- Phase 5: output projection ----------------
            with tc.tile_pool(name="outp", bufs=3) as outp:
                for sc in range(MT):
                    wps = ps.tile([128, QB], F32, tag="ps")
                    for nb in range(2):
                        n5 = slice(nb * 512, (nb + 1) * 512)
                        for h in range(4):
                            nc.tensor.matmul(
                                wps[:, n5],
                                lhsT=outT[0:64, h, sc * 128:(sc + 1) * 128],
                                rhs=wo_sb[0:64, h, n5],
                                start=(h == 0), stop=(h == 3))
                    co = outp.tile([128, DIM], F32, tag="co")
                    nc.scalar.copy(co, wps)
                    nc.sync.dma_start(out=out_part[sc * 128:(sc + 1) * 128, :], in_=co)

    nc.compile()
    return nc


def _rope_perm_cols():
    """Column permutation of the 256-wide W slice for one core's 4 heads.

    Chunk c (0,1) holds local heads 2c, 2c+1 as rows
    [hA_even(32) | hA_odd(32) | hB_even(32) | hB_odd(32)].
    """
    cols = []
    for c in range(2):
        for j2 in range(2):          # which head within the chunk
            head = 2 * c + j2
            for blk in range(2):     # 0: even dims, 1: odd dims
                for i in range(32):
                    cols.append(head * 64 + 2 * i + blk)
    return np.array(cols)


def _cos_sin_tables():
    inv_freq = 1.0 / (ROPE_THETA ** (np.arange(0, HEAD_DIM, 2, dtype=np.float64)
                                     / HEAD_DIM))          # [32]
    ang = np.arange(S, dtype=np.float64)[None, :] * inv_freq[:, None]  # [32, S]
    cos32 = np.cos(ang).astype(np.float32)
    sin32 = np.sin(ang).astype(np.float32)
    cosT = np.tile(cos32, (4, 1))                           # [128, S]
    # sign: +sin at even-dim rows (blocks 0, 2), -sin at odd-dim rows (1, 3)
    sinT = np.concatenate([sin32, -sin32, sin32, -sin32], axis=0)
    return np.ascontiguousarray(cosT), np.ascontiguousarray(sinT)


def _tile_xT(xT):
    # [1024, 2048] -> [4 sblk, 128 part, 8 kc, 512]
    return np.ascontiguousarray(
        xT.reshape(KC, 128, 4, 512).transpose(2, 1, 0, 3))


def _tile_vT(vT):
    # [1024, 2048] -> [16 sc, 128 part, 8 kc, 128]
    return np.ascontiguousarray(
        vT.reshape(KC, 128, MT, 128).transpose(2, 1, 0, 3))


def _tile_w(w):
    # [1024, 256] -> [128, 8, 256]
    return np.ascontiguousarray(w.reshape(KC, 128, 256).transpose(1, 0, 2))


def _tile_mask(maskT_bf16):
    # [2048, 2048] -> [128, 16 m, 2048]
    return np.ascontiguousarray(
        maskT_bf16.reshape(MT, 128, S).transpose(1, 0, 2))


def kernel(q, k, v, mask, Wq, Wk, Wv, Wo, bo):
    global _BUILT
    if _BUILT is None:
        _BUILT = build_bass()
    nc = _BUILT

    q = np.asarray(q, np.float32)
    k = np.asarray(k, np.float32)
    v = np.asarray(v, np.float32)
    Wq = np.asarray(Wq, np.float32)
    Wk = np.asarray(Wk, np.float32)
    Wv = np.asarray(Wv, np.float32)
    Wo = np.asarray(Wo, np.float32)
    bo = np.asarray(bo, np.float32)
    mask = np.asarray(mask)

    cosT, sinT = _cos_sin_tables()
    ones64 = np.ones((1, 64), np.float32)
    perm = _rope_perm_cols()
    qTb = [_tile_xT(q[b].T) for b in range(2)]
    kTb = [_tile_xT(k[b].T) for b in range(2)]
    vTb = [_tile_vT(v[b].T) for b in range(2)]
    maskTb = [_tile_mask(mask[b, 0].T.astype(ml_dtypes.bfloat16)) for b in range(2)]

    in_maps = []
    for c in range(N_CORES):
        b = c // 4
        head_base = (c % 4) * 4
        cols = slice(head_base * 64, head_base * 64 + 256)
        in_maps.append({
            "qT": qTb[b], "kT": kTb[b], "vT": vTb[b],
            "wq": _tile_w(Wq[:, cols][:, perm]),
            "wk": _tile_w(Wk[:, cols][:, perm]),
            "wv": _tile_w(Wv[:, cols]),
            "wo": np.ascontiguousarray(
                Wo[cols, :].reshape(4, 64, DIM).transpose(1, 0, 2)),
            "cosT": cosT, "sinT": sinT,
            "maskT": maskTb[b], "ones64": ones64,
        })

    kernel._last_in_maps = in_maps
    res = run_bass_kernel_spmd(nc, in_maps, core_ids=list(range(N_CORES)))
    out = np.zeros((2, S, DIM), np.float32)
    for c in range(N_CORES):
        out[c // 4] += res.results[c]["out_part"]
    out += bo[None, None, :]
    return out



# revision 6
# speedup vs baseline: 1.2996x; 1.2996x over previous
"""Multi-head attention (RoPE + mask + softmax) Trainium2 Bass kernel.

Sharding: 8 cores = 2 batches x 4 head-groups. Core c handles batch c//4,
local heads 4*(c%4) .. +4 (tensor-parallel on heads; Wq/Wk/Wv column-sharded,
Wo row-sharded; per-core partial outputs summed on host).

v2 (restructured for engine overlap, all-bf16 matmul operands):
  - All matmul operands bf16 (host-cast); PSUM accumulation f32.
  - Scores h2=0/h2=1 emitted back-to-back: K=64 row-tiled pairs run
    concurrently in the PE array (rows 0-63 / 64-127).
  - exp on ACT only; mask-mul on DVE (bf16 SBUF 2x mode); denominator via
    ones-column row of the AV psum -> DVE reciprocal_approx_fast ->
    gpsimd partition_broadcast -> fused DVE normalize (no DRAM bounce).
  - Single 8-bank PSUM plan: tag "work" [128,1024] f32 x2 (scores / proj /
    outproj share slots), tag "avp" [65,1024] f32 x2.
  - v-proj + q-proj(qb1) interleaved into attention qb0; outproj(qb0)
    interleaved into attention qb1. out_part stored bf16, summed on host.
"""
import sys
sys.path.insert(0, '/opt/trn_rl_repo')
import math
import numpy as np
import ml_dtypes

import concourse.bass as bass
import concourse.mybir as mybir
import concourse.tile as tile
from concourse import bacc
from concourse.bass_utils import run_bass_kernel_spmd

F32 = mybir.dt.float32
BF16 = mybir.dt.bfloat16

S = 2048
DIM = 1024
HEAD_DIM = 64
N_CORES = 8
KC = DIM // 128          # 8 contraction chunks for projections
MT = S // 128            # 16 k-chunks in attention
QB = 1024                # q-block width
NQB = S // QB            # 2
ROPE_THETA = 10000.0

_BUILT = None


def build_bass(dbg=False):
    nc = bacc.Bacc("TRN2", target_bir_lowering=False, debug=False)

    qT = nc.dram_tensor("qT", [4, 128, KC, 512], BF16, kind="ExternalInput").ap()
    kT = nc.dram_tensor("kT", [4, 128, KC, 512], BF16, kind="ExternalInput").ap()
    vT = nc.dram_tensor("vT", [MT, 128, KC, 128], BF16, kind="ExternalInput").ap()
    wq = nc.dram_tensor("wq", [128, KC, 256], BF16, kind="ExternalInput").ap()
    wk = nc.dram_tensor("wk", [128, KC, 256], BF16, kind="ExternalInput").ap()
    wv = nc.dram_tensor("wv", [128, KC, 256], BF16, kind="ExternalInput").ap()
    wo = nc.dram_tensor("wo", [64, 4, DIM], BF16, kind="ExternalInput").ap()
    cosT = nc.dram_tensor("cosT", [128, S], F32, kind="ExternalInput").ap()
    sinT = nc.dram_tensor("sinT", [128, S], F32, kind="ExternalInput").ap()
    maskT = nc.dram_tensor("maskT", [128, MT, S], BF16, kind="ExternalInput").ap()
    out_part = nc.dram_tensor("out_part", [S, DIM], BF16, kind="ExternalOutput").ap()
    if dbg:
        qhT_d = nc.dram_tensor("qhT_d", [128, 2, S], BF16, kind="ExternalOutput").ap()
        khT_d = nc.dram_tensor("khT_d", [128, 2, S], BF16, kind="ExternalOutput").ap()
        vh_d = nc.dram_tensor("vh_d", [128, MT, 4 * 65], BF16, kind="ExternalOutput").ap()
        at_d = nc.dram_tensor("at_d", [128, 8, QB], BF16, kind="ExternalOutput").ap()
        outT_d = nc.dram_tensor("outT_d", [64, 4, S], BF16, kind="ExternalOutput").ap()

    with tile.TileContext(nc) as tc:
        with tc.tile_pool(name="persist", bufs=1) as persist, \
             tc.tile_pool(name="ps", bufs=2, space="PSUM") as ps, \
             tc.tile_pool(name="xts", bufs=2) as xts, \
             tc.tile_pool(name="rope", bufs=2) as rope, \
             tc.tile_pool(name="attn", bufs=3) as attnp, \
             tc.tile_pool(name="dn", bufs=2) as dnp, \
             tc.tile_pool(name="outp", bufs=2) as outp:

            qhT = persist.tile([128, 2, S], BF16)     # [chunk-part, hp, s]
            khT = persist.tile([128, 2, S], BF16)
            vh = persist.tile([128, MT, 4 * 65], BF16)
            outT = persist.tile([64, 4, S], BF16)
            wo_sb = persist.tile([64, 4, DIM], BF16)
            wq_sb = persist.tile([128, KC, 256], BF16)
            wk_sb = persist.tile([128, KC, 256], BF16)
            wv_sb = persist.tile([128, KC, 256], BF16)
            cos_sb = persist.tile([128, S], F32)
            sin_sb = persist.tile([128, S], F32)
            mk = persist.tile([128, MT, S], BF16)

            # ---- input DMAs (sync queue: weights + x tiles, in need order) ----
            nc.sync.dma_start(out=wk_sb, in_=wk)
            nc.sync.dma_start(out=wq_sb, in_=wq)
            nc.sync.dma_start(out=wv_sb, in_=wv)
            # scalar queue: rope tables, wo, mask
            nc.scalar.dma_start(out=cos_sb, in_=cosT)
            nc.scalar.dma_start(out=sin_sb, in_=sinT)
            nc.scalar.dma_start(out=wo_sb, in_=wo)
            for mq in range(4):
                nc.scalar.dma_start(out=mk[:, mq * 4:(mq + 1) * 4, :],
                                    in_=maskT[:, mq * 4:(mq + 1) * 4, :])
            # ones column for the denominator rows of vh
            nc.vector.memset(
                vh.rearrange("p m (h x) -> p m h x", x=65)[:, :, :, 64:65], 1.0)

            def proj_rope(xdram, w_sb, dstT, sblk, hp):
                """One (sblk, hp) projection unit with fused RoPE eviction."""
                tag = f"x{sblk}"
                if not hasattr(proj_rope, "_x"):
                    proj_rope._x = {}
                key = (id(xdram), sblk)
                if key not in proj_rope._x:
                    x_sb = xts.tile([128, KC, 512], BF16, tag="xts")
                    nc.sync.dma_start(out=x_sb, in_=xdram[sblk])
                    proj_rope._x[key] = x_sb
                x_sb = proj_rope._x[key]
                ss = slice(sblk * 512, (sblk + 1) * 512)
                psum = ps.tile([128, 512], F32, tag="work")
                for kc in range(KC):
                    nc.tensor.matmul(
                        psum,
                        lhsT=w_sb[:, kc, hp * 128:(hp + 1) * 128],
                        rhs=x_sb[:, kc, :],
                        start=(kc == 0), stop=(kc == KC - 1))
                t = rope.tile([128, 512], F32, tag="t")
                u = rope.tile([128, 512], F32, tag="u")
                nc.vector.tensor_mul(t, psum, cos_sb[:, ss])
                nc.vector.tensor_mul(u, psum, sin_sb[:, ss])
                us = rope.tile([128, 512], F32, tag="us")
                for blk in range(4):
                    a, b2 = blk * 32, (blk ^ 1) * 32
                    nc.sync.dma_start(out=us[a:a + 32, :], in_=u[b2:b2 + 32, :])
                nc.vector.tensor_add(dstT[:, hp, ss], t, us)

            def vproj(sc):
                v_sb = xts.tile([128, KC, 128], BF16, tag="vts")
                nc.sync.dma_start(out=v_sb, in_=vT[sc])
                psum = ps.tile([128, 256], F32, tag="work")
                for kc in range(KC):
                    nc.tensor.matmul(
                        psum, lhsT=v_sb[:, kc, :], rhs=wv_sb[:, kc, :],
                        start=(kc == 0), stop=(kc == KC - 1))
                nc.vector.tensor_copy(
                    vh[:, sc, :].rearrange("p (h x) -> p h x", x=65)[:, :, 0:64],
                    psum.rearrange("p (h x) -> p h x", x=64))

            def outproj(sc):
                """Output projection for one 128-row s-chunk (both n halves)."""
                oc = outp.tile([128, DIM], BF16, tag="oc")
                for nb in range(2):
                    n5 = slice(nb * 512, (nb + 1) * 512)
                    wps = ps.tile([128, 512], F32, tag="work")
                    for h in range(4):
                        nc.tensor.matmul(
                            wps,
                            lhsT=outT[0:64, h, sc * 128:(sc + 1) * 128],
                            rhs=wo_sb[0:64, h, n5],
                            start=(h == 0), stop=(h == 3))
                    nc.scalar.copy(oc[:, n5], wps)
                nc.sync.dma_start(out=out_part[sc * 128:(sc + 1) * 128, :], in_=oc)

            # fillers: PE work units to interleave into attention m-loops
            fillers = []
            for sc in range(8, MT):
                fillers.append(lambda sc=sc: vproj(sc))
            for sblk in (2, 3):
                for hp in range(2):
                    fillers.append(
                        lambda sblk=sblk, hp=hp: proj_rope(qT, wq_sb, qhT, sblk, hp))

            def attention_unit(qb, hp, interleave):
                """Attention for (q-block, head-pair): 16 m-chunks + normalize."""
                qs = slice(qb * QB, (qb + 1) * QB)
                avp = [ps.tile([65, QB], F32, tag="avp", name=f"avp{qb}_{hp}_{i}")
                       for i in range(2)]
                sps_t = [None, None]
                for m in range(MT):
                    if interleave and m % 2 == 0 and fillers:
                        fillers.pop(0)()
                    # paired score matmuls: h2=0 rows 0-63, h2=1 rows 64-127
                    for h2 in range(2):
                        sps_t[h2] = ps.tile([128, QB], F32, tag="work",
                                            name=f"sps_{qb}_{hp}_{m}_{h2}")
                    for q2 in range(2):
                        q5 = slice(q2 * 512, (q2 + 1) * 512)
                        for h2 in range(2):
                            hb = slice(h2 * 64, (h2 + 1) * 64)
                            nc.tensor.matmul(
                                sps_t[h2][:, q5],
                                lhsT=khT[hb, hp, m * 128:(m + 1) * 128],
                                rhs=qhT[hb, hp, qs][:, q2 * 512:(q2 + 1) * 512],
                                start=True, stop=True)
                    for h2 in range(2):
                        at = attnp.tile([128, QB], BF16, tag="at")
                        nc.scalar.activation(
                            at, sps_t[h2], mybir.ActivationFunctionType.Exp,
                            scale=1.0 / math.sqrt(HEAD_DIM))
                        atm = attnp.tile([128, QB], BF16, tag="atm")
                        nc.vector.tensor_mul(atm, at, mk[:, m, qs])
                        if dbg and qb == 0 and hp == 0 and m < 4:
                            nc.sync.dma_start(out=at_d[:, 2 * m + h2, :], in_=atm)
                        h = 2 * hp + h2
                        for q2 in range(2):
                            q5 = slice(q2 * 512, (q2 + 1) * 512)
                            nc.tensor.matmul(
                                avp[h2][:, q5],
                                lhsT=vh[:, m, h * 65:(h + 1) * 65],
                                rhs=atm[:, q5],
                                start=(m == 0), stop=(m == MT - 1))
                # normalize: recip of denominator row, broadcast, fused evict
                for h2 in range(2):
                    h = 2 * hp + h2
                    rdn = dnp.tile([1, QB], F32, tag="rdn")
                    nc.vector.reciprocal(rdn, avp[h2][64:65, :])
                    rdnb = dnp.tile([64, QB], F32, tag="rdnb")
                    nc.gpsimd.partition_broadcast(rdnb, rdn)
                    nc.vector.tensor_mul(outT[0:64, h, qs], avp[h2][0:64, :], rdnb)

            # ---------------- emission schedule ----------------
            for sblk in range(4):
                for hp in range(2):
                    proj_rope(kT, wk_sb, khT, sblk, hp)
            for sblk in (0, 1):
                for hp in range(2):
                    proj_rope(qT, wq_sb, qhT, sblk, hp)
            for sc in range(8):
                vproj(sc)

            attention_unit(0, 0, interleave=True)
            attention_unit(0, 1, interleave=True)
            # qb1 attention with outproj(qb0) interleaved
            ofill = list(range(8))

            def attention_unit_op(qb, hp, oscs):
                qs = slice(qb * QB, (qb + 1) * QB)
                avp = [ps.tile([65, QB], F32, tag="avp", name=f"avp{qb}_{hp}_{i}")
                       for i in range(2)]
                sps_t = [None, None]
                oi = 0
                for m in range(MT):
                    if m % 2 == 0 and oi < len(oscs):
                        outproj(oscs[oi])
                        oi += 1
                    for h2 in range(2):
                        sps_t[h2] = ps.tile([128, QB], F32, tag="work",
                                            name=f"sps_{qb}_{hp}_{m}_{h2}")
                    for q2 in range(2):
                        q5 = slice(q2 * 512, (q2 + 1) * 512)
                        for h2 in range(2):
                            hb = slice(h2 * 64, (h2 + 1) * 64)
                            nc.tensor.matmul(
                                sps_t[h2][:, q5],
                                lhsT=khT[hb, hp, m * 128:(m + 1) * 128],
                                rhs=qhT[hb, hp, qs][:, q2 * 512:(q2 + 1) * 512],
                                start=True, stop=True)
                    for h2 in range(2):
                        at = attnp.tile([128, QB], BF16, tag="at")
                        nc.scalar.activation(
                            at, sps_t[h2], mybir.ActivationFunctionType.Exp,
                            scale=1.0 / math.sqrt(HEAD_DIM))
                        atm = attnp.tile([128, QB], BF16, tag="atm")
                        nc.vector.tensor_mul(atm, at, mk[:, m, qs])
                        h = 2 * hp + h2
                        for q2 in range(2):
                            q5 = slice(q2 * 512, (q2 + 1) * 512)
                            nc.tensor.matmul(
                                avp[h2][:, q5],
                                lhsT=vh[:, m, h * 65:(h + 1) * 65],
                                rhs=atm[:, q5],
                                start=(m == 0), stop=(m == MT - 1))
                while oi < len(oscs):
                    outproj(oscs[oi])
                    oi += 1
                for h2 in range(2):
                    h = 2 * hp + h2
                    rdn = dnp.tile([1, QB], F32, tag="rdn")
                    nc.vector.reciprocal(rdn, avp[h2][64:65, :])
                    rdnb = dnp.tile([64, QB], F32, tag="rdnb")
                    nc.gpsimd.partition_broadcast(rdnb, rdn)
                    nc.vector.tensor_mul(outT[0:64, h, qs], avp[h2][0:64, :], rdnb)

            attention_unit_op(1, 0, list(range(0, 4)))
            attention_unit_op(1, 1, list(range(4, 8)))
            for sc in range(8, MT):
                outproj(sc)
            if dbg:
                nc.sync.dma_start(out=qhT_d, in_=qhT)
                nc.sync.dma_start(out=khT_d, in_=khT)
                nc.sync.dma_start(out=vh_d, in_=vh)
                nc.sync.dma_start(out=outT_d, in_=outT)

    nc.compile()
    return nc


def _rope_perm_cols():
    """Column permutation of the 256-wide W slice for one core's 4 heads.

    Chunk c (0,1) holds local heads 2c, 2c+1 as rows
    [hA_even(32) | hA_odd(32) | hB_even(32) | hB_odd(32)].
    """
    cols = []
    for c in range(2):
        for j2 in range(2):          # which head within the chunk
            head = 2 * c + j2
            for blk in range(2):     # 0: even dims, 1: odd dims
                for i in range(32):
                    cols.append(head * 64 + 2 * i + blk)
    return np.array(cols)


def _cos_sin_tables():
    inv_freq = 1.0 / (ROPE_THETA ** (np.arange(0, HEAD_DIM, 2, dtype=np.float64)
                                     / HEAD_DIM))          # [32]
    ang = np.arange(S, dtype=np.float64)[None, :] * inv_freq[:, None]  # [32, S]
    cos32 = np.cos(ang).astype(np.float32)
    sin32 = np.sin(ang).astype(np.float32)
    cosT = np.tile(cos32, (4, 1))                           # [128, S]
    # sign: +sin at even-dim rows (blocks 0, 2), -sin at odd-dim rows (1, 3)
    sinT = np.concatenate([sin32, -sin32, sin32, -sin32], axis=0)
    return np.ascontiguousarray(cosT), np.ascontiguousarray(sinT)


def _tile_xT(xT):
    # [1024, 2048] -> [4 sblk, 128 part, 8 kc, 512]
    return np.ascontiguousarray(
        xT.reshape(KC, 128, 4, 512).transpose(2, 1, 0, 3))


def _tile_vT(vT):
    # [1024, 2048] -> [16 sc, 128 part, 8 kc, 128]
    return np.ascontiguousarray(
        vT.reshape(KC, 128, MT, 128).transpose(2, 1, 0, 3))


def _tile_w(w):
    # [1024, 256] -> [128, 8, 256]
    return np.ascontiguousarray(w.reshape(KC, 128, 256).transpose(1, 0, 2))


def _tile_mask(maskT_bf16):
    # [2048, 2048] -> [128, 16 m, 2048]
    return np.ascontiguousarray(
        maskT_bf16.reshape(MT, 128, S).transpose(1, 0, 2))


def kernel(q, k, v, mask, Wq, Wk, Wv, Wo, bo):
    global _BUILT
    if _BUILT is None:
        _BUILT = build_bass()
    nc = _BUILT

    BF = ml_dtypes.bfloat16
    q = np.asarray(q, np.float32)
    k = np.asarray(k, np.float32)
    v = np.asarray(v, np.float32)
    Wq = np.asarray(Wq, np.float32)
    Wk = np.asarray(Wk, np.float32)
    Wv = np.asarray(Wv, np.float32)
    Wo = np.asarray(Wo, np.float32)
    bo = np.asarray(bo, np.float32)
    mask = np.asarray(mask)

    cosT, sinT = _cos_sin_tables()
    perm = _rope_perm_cols()
    qTb = [_tile_xT(q[b].T.astype(BF)) for b in range(2)]
    kTb = [_tile_xT(k[b].T.astype(BF)) for b in range(2)]
    vTb = [_tile_vT(v[b].T.astype(BF)) for b in range(2)]
    maskTb = [_tile_mask(mask[b, 0].T.astype(BF)) for b in range(2)]

    in_maps = []
    for c in range(N_CORES):
        b = c // 4
        head_base = (c % 4) * 4
        cols = slice(head_base * 64, head_base * 64 + 256)
        in_maps.append({
            "qT": qTb[b], "kT": kTb[b], "vT": vTb[b],
            "wq": _tile_w(Wq[:, cols][:, perm].astype(BF)),
            "wk": _tile_w(Wk[:, cols][:, perm].astype(BF)),
            "wv": _tile_w(Wv[:, cols].astype(BF)),
            "wo": np.ascontiguousarray(
                Wo[cols, :].reshape(4, 64, DIM).transpose(1, 0, 2).astype(BF)),
            "cosT": cosT, "sinT": sinT,
            "maskT": maskTb[b],
        })

    kernel._last_in_maps = in_maps
    res = run_bass_kernel_spmd(nc, in_maps, core_ids=list(range(N_CORES)))
    out = np.zeros((2, S, DIM), np.float32)
    for c in range(N_CORES):
        out[c // 4] += res.results[c]["out_part"].astype(np.float32)
    out += bo[None, None, :]
    return out
